# revision 1
# baseline (speedup 1.0000x reference)
"""Trainium2 Bass kernel for nn_Chamfer_Loss (chamfer + mesh regularizers).

Strategy (8 NeuronCores, SPMD, per-core data differs):
  - Chamfer (pos + velocity, both directions) = 8 "orientation tasks", one per
    core: core computes row-maxes of t_ij = q_i.k_j - 0.5|q_i|^2 - 0.5|k_j|^2
    (= -d_ij/2) via K=5 augmented matmuls (fp16 hi/lo 3-pass for ~fp32
    accuracy), reducing each PSUM group on VectorE.  min_j d_ij = relu(-2 max_j
    t_ij).
  - Mesh losses (edge / cot-laplacian / normal consistency) are sharded 1/8 per
    core over faces/edges/pairs; vertex gathers via indirect DMA; the laplacian
    scatter-add uses host-precomputed collision-free expanded slots
    (row = vertex*SLOT + occurrence) + DMA compute_op=add, then a dense on-chip
    reduction back to per-vertex partial sums.
  - Host combines 8 cores' partial scalars / partial vertex sums into the final
    scalar loss (O(N) work only).
"""

import math

import numpy as np

import concourse.bass as bass
import concourse.bacc as bacc
import concourse.mybir as mybir
import concourse.tile as tile

MM_DTYPE = "bf16"  # "f16" | "bf16"
CHUNKW = 512  # matmul moving width (walrus caps moving dim at 512)
REDUCE_MODE = "bf16max"  # "direct" (reduce each PSUM group) | "bf16max" (ACT cast + DVE 2x max)

AluOp = mybir.AluOpType
ActFn = mybir.ActivationFunctionType
F32 = mybir.dt.float32
F16 = mybir.dt.float16
BF16 = mybir.dt.bfloat16
I32 = mybir.dt.int32


def _mm_dt():
    return F16 if MM_DTYPE == "f16" else BF16


def _np_mm_dt():
    import ml_dtypes
    import numpy as _np

    return _np.float16 if MM_DTYPE == "f16" else ml_dtypes.bfloat16

P = 128
NCORES = 8
W_EDGE, W_LAP, W_NORMAL, W_VEL = 0.5, 0.05, 0.01, 10.0
BIGNEG = 30000.0  # key-padding bias: t_pad <= -BIGNEG + small
AREA_EPS = 1.6e-11  # 16 * 1e-12 (Heron discriminant clamp, matches reference)

FULL_DIMS = dict(n=8281, f=16200, e=24480, pr=24120, slot=8)


def _cfg(dims):
    n = dims["n"]
    rt = -(-n // P)
    cc = -(-n // 512)
    fpc = -(-dims["f"] // NCORES)
    epc = -(-dims["e"] // NCORES)
    ppc = -(-dims["pr"] // NCORES)
    cfg = dict(
        n=n,
        f=dims["f"],
        e=dims["e"],
        pr=dims["pr"],
        slot=dims["slot"],
        RT=rt,
        CC=cc,
        NQP=rt * P,
        NKP=n,
        FPC=fpc,
        EPC=epc,
        PPC=ppc,
        FK=-(-fpc // P),
        EK=-(-epc // P),
        PK=-(-ppc // P),
    )
    cfg["VROWS"] = cfg["NQP"]  # >= n, multiple of 128
    cfg["ACCROWS"] = cfg["VROWS"] * cfg["slot"]  # 8-channel rows
    cfg["ACCFLAT"] = cfg["ACCROWS"] * 8
    # chunk list (<=CHUNKW each) and groups of <=2048 psum columns per reduce
    chunks = []
    o = 0
    while o < n:
        w = min(CHUNKW, n - o)
        chunks.append((o, w))
        o += w
    per = max(1, 2048 // CHUNKW)
    groups = [chunks[i : i + per] for i in range(0, len(chunks), per)]
    cfg["GROUPS"] = groups
    return cfg


# --------------------------------------------------------------------------
# device program
# --------------------------------------------------------------------------


def build_program(cfg, repeat=1, phases=("chamfer", "mesh")):
    nc = bacc.Bacc("TRN2", target_bir_lowering=False, debug=False, num_devices=NCORES)

    RT, CC, NQP, NKP = cfg["RT"], cfg["CC"], cfg["NQP"], cfg["NKP"]
    FK, EK, PK, SLOT = cfg["FK"], cfg["EK"], cfg["PK"], cfg["slot"]
    VROWS = cfg["VROWS"]

    # ---- I/O ----
    MMDT = _mm_dt()
    lhs15 = nc.dram_tensor("lhs15", [15, NQP], MMDT, kind="ExternalInput")
    rhs15 = nc.dram_tensor("rhs15", [15, NKP], MMDT, kind="ExternalInput")
    qmask = nc.dram_tensor("qmask", [P, RT], F32, kind="ExternalInput")
    vtab = nc.dram_tensor("vtab", [cfg["n"], 8], F32, kind="ExternalInput")
    fidx = [
        nc.dram_tensor(f"fidx{s}", [P, FK], I32, kind="ExternalInput") for s in range(3)
    ]
    sidx = [
        nc.dram_tensor(f"sidx{s}", [P, FK], I32, kind="ExternalInput") for s in range(3)
    ]
    eidx = [
        nc.dram_tensor(f"eidx{s}", [P, EK], I32, kind="ExternalInput") for s in range(2)
    ]
    emask = nc.dram_tensor("emask", [P, EK], F32, kind="ExternalInput")
    pidx = [
        nc.dram_tensor(f"pidx{s}", [P, PK], I32, kind="ExternalInput") for s in range(4)
    ]
    pmask = nc.dram_tensor("pmask", [P, PK], F32, kind="ExternalInput")
    oscal = nc.dram_tensor("oscal", [8, 1], F32, kind="ExternalOutput")
    ovsum = nc.dram_tensor("ovsum", [VROWS, 8], F32, kind="ExternalOutput")

    with tile.TileContext(nc) as tc:
        with (
            tc.tile_pool(name="const", bufs=1) as cp,
            tc.tile_pool(name="work", bufs=2) as wp,
            tc.tile_pool(name="dram", bufs=1, space="DRAM") as dp,
        ):
            accs = [
                dp.tile([cfg["ACCFLAT"]], F32, tag=f"acc{s}", name=f"acc{s}")
                for s in range(3)
            ]

            if repeat > 1:
                rep_ctx = tc.For_i(0, repeat, 1)
                rep_ctx.__enter__()

            # ---- load chamfer operands ----
            lhs15_t = cp.tile([15, NQP], MMDT, tag="lhs15")
            rhs15_t = cp.tile([15, NKP], MMDT, tag="rhs15")
            qmask_t = cp.tile([P, RT], F32, tag="qmask")
            for t_, d_ in (
                (lhs15_t, lhs15),
                (rhs15_t, rhs15),
                (qmask_t, qmask),
            ):
                nc.sync.dma_start(out=t_[:], in_=d_.ap())

            # ---- zero the lap accumulator (8.5MB) ----
            zrow = 2048
            zt = cp.tile([P, zrow], F32, tag="zero")
            nc.gpsimd.memset(zt[:], 0.0)
            if "mesh" in phases:
                for a_ in accs:
                    accz = a_[:].rearrange("(a b) -> a b", b=zrow)
                    nzr = accz.shape[0]
                    for d in range(0, nzr, P):
                        h = min(P, nzr - d)
                        nc.sync.dma_start(out=accz[d : d + h, :], in_=zt[:h, :])

            # ---- chamfer: row-maxes of t ----
            do_chamfer = "chamfer" in phases
            do_mesh = "mesh" in phases
            mm_only = "mm_only" in phases
            red_only = "red_only" in phases
            npass = 1 if "p1" in phases else 3
            rmB = cp.tile([P, RT], F32, tag="rmB")
            if not do_chamfer:
                nc.gpsimd.memset(rmB[:], 0.0)
            with tc.tile_pool(name="psum", bufs=2, space="PSUM") as pp:
                use_bf16max = REDUCE_MODE == "bf16max"
                for rt_i in range(RT if do_chamfer else 0):
                    lw = lhs15_t[:, rt_i * P : (rt_i + 1) * P]
                    rm5 = wp.tile([P, 8], F32, tag="rm5")
                    bigs = []
                    ncols = 0
                    for gi, grp in enumerate(cfg["GROUPS"]):
                        ps = pp.tile([P, 2048], F32, tag="psg")
                        gw = sum(cw for _, cw in grp)
                        if red_only:
                            nc.tensor.matmul(
                                out=ps[:16, :16], lhsT=lw[:, :16], rhs=rhs15_t[:, :16],
                                start=True, stop=True,
                            )
                        else:
                            pl0 = 0
                            for co, cw in grp:
                                nc.tensor.matmul(
                                    out=ps[:, pl0 : pl0 + cw],
                                    lhsT=lw,
                                    rhs=rhs15_t[:, co : co + cw],
                                    start=True,
                                    stop=True,
                                )
                                pl0 += cw
                        if mm_only:
                            continue
                        if use_bf16max and gw == 2048:
                            sb = wp.tile(
                                [P, 2048], BF16, tag=f"sbg{len(bigs) % 4}",
                                name=f"sbg{len(bigs) % 4}",
                            )
                            nc.scalar.activation(out=sb[:], in_=ps[:], func=ActFn.Copy)
                            bigs.append(sb)
                        else:
                            nc.vector.tensor_reduce(
                                out=rm5[:, ncols : ncols + 1], in_=ps[:, :gw],
                                axis=mybir.AxisListType.X, op=AluOp.max,
                            )
                            ncols += 1
                    if mm_only:
                        nc.gpsimd.memset(rm5[:], 0.0)
                        ncols = 1
                    if bigs:
                        red_src = bigs[0]
                        if len(bigs) > 1:
                            accT = wp.tile([P, 2048], BF16, tag="accT")
                            nc.vector.tensor_tensor(
                                out=accT[:], in0=bigs[0][:], in1=bigs[1][:], op=AluOp.max
                            )
                            for b_ in bigs[2:]:
                                nc.vector.tensor_tensor(
                                    out=accT[:], in0=accT[:], in1=b_[:], op=AluOp.max
                                )
                            red_src = accT
                        nc.vector.tensor_reduce(
                            out=rm5[:, ncols : ncols + 1], in_=red_src[:],
                            axis=mybir.AxisListType.X, op=AluOp.max,
                        )
                        ncols += 1
                    nc.vector.tensor_reduce(
                        out=rmB[:, rt_i : rt_i + 1], in_=rm5[:, :ncols],
                        axis=mybir.AxisListType.X, op=AluOp.max,
                    )

            # chamfer partial scalar: sum over valid rows of relu(-2 * rowmax)
            scal8 = cp.tile([P, 8], F32, tag="scal8")
            nc.gpsimd.memset(scal8[:], 0.0)
            chtmp = cp.tile([P, RT], F32, tag="chtmp")
            nc.vector.tensor_scalar(
                out=chtmp[:], in0=rmB[:], scalar1=-2.0, scalar2=0.0,
                op0=AluOp.mult, op1=AluOp.max,
            )
            nc.vector.tensor_tensor(out=chtmp[:], in0=chtmp[:], in1=qmask_t[:], op=AluOp.mult)
            nc.vector.tensor_reduce(
                out=scal8[:, 0:1], in_=chtmp[:], axis=mybir.AxisListType.X, op=AluOp.add
            )

            # ---- mesh: gathers ----
            def gather(idx_dram, K, tag):
                # one indirect DMA per 128 indices (multi-index offset APs are
                # not HW-consistent), offsets [P, 1] like tile_scatter_add
                it = cp.tile([P, K], I32, tag=tag + "_i", name=tag + "_i")
                nc.sync.dma_start(out=it[:], in_=idx_dram.ap())
                gt = cp.tile([P, K, 8], F32, tag=tag + "_g", name=tag + "_g")
                for k in range(K):
                    nc.gpsimd.indirect_dma_start(
                        out=gt[:, k, :],
                        out_offset=None,
                        in_=vtab.ap(),
                        in_offset=bass.IndirectOffsetOnAxis(ap=it[:, k : k + 1], axis=0),
                    )
                return gt

            if not do_mesh:
                nc.gpsimd.memset(vsum_zero := cp.tile([P, VROWS // P, 8], F32, tag="vsum0", name="vsum0"), 0.0)
                nc.sync.dma_start(out=ovsum.ap().rearrange("(vb p) c -> p vb c", p=P), in_=vsum_zero[:])
            fv = [gather(fidx[s], FK, f"fv{s}") for s in range(3) if do_mesh]
            ev = [gather(eidx[s], EK, f"ev{s}") for s in range(2) if do_mesh]
            pv = [gather(pidx[s], PK, f"pv{s}") for s in range(4) if do_mesh]

            emask_t = cp.tile([P, EK], F32, tag="emask")
            nc.sync.dma_start(out=emask_t[:], in_=emask.ap())
            pmask_t = cp.tile([P, PK], F32, tag="pmask")
            nc.sync.dma_start(out=pmask_t[:], in_=pmask.ap())

            # ---- edge loss ----
            for b in ((0, 1) if do_mesh else ()):
                ch = slice(4 * b, 4 * b + 3)
                ed = wp.tile([P, EK, 3], F32, tag="ed")
                nc.vector.tensor_tensor(
                    out=ed[:], in0=ev[0][:, :, ch], in1=ev[1][:, :, ch], op=AluOp.subtract
                )
                nc.vector.tensor_tensor(out=ed[:], in0=ed[:], in1=ed[:], op=AluOp.mult)
                es = wp.tile([P, EK], F32, tag="es")
                nc.vector.tensor_reduce(
                    out=es[:], in_=ed[:], axis=mybir.AxisListType.X, op=AluOp.add
                )
                nc.vector.tensor_tensor(out=es[:], in0=es[:], in1=emask_t[:], op=AluOp.mult)
                nc.vector.tensor_reduce(
                    out=scal8[:, 1 + b : 2 + b], in_=es[:],
                    axis=mybir.AxisListType.X, op=AluOp.add,
                )

            # ---- cot laplacian: per-face weights + scatter rows ----
            sval = [cp.tile([P, FK, 8], F32, tag=f"sval{s}", name=f"sval{s}") for s in range(3)]
            for b in ((0, 1) if do_mesh else ()):
                ch = slice(4 * b, 4 * b + 3)
                v0, v1, v2 = (fv[s][:, :, ch] for s in range(3))
                e12 = wp.tile([P, FK, 3], F32, tag="e12")
                e02 = wp.tile([P, FK, 3], F32, tag="e02")
                e01 = wp.tile([P, FK, 3], F32, tag="e01")
                nc.vector.tensor_tensor(out=e12[:], in0=v1, in1=v2, op=AluOp.subtract)
                nc.vector.tensor_tensor(out=e02[:], in0=v0, in1=v2, op=AluOp.subtract)
                nc.vector.tensor_tensor(out=e01[:], in0=v0, in1=v1, op=AluOp.subtract)
                sq = wp.tile([P, FK, 3], F32, tag="sq")
                A2 = wp.tile([P, FK], F32, tag="A2")
                B2 = wp.tile([P, FK], F32, tag="B2")
                C2 = wp.tile([P, FK], F32, tag="C2")
                for dsq, ee in ((A2, e12), (B2, e02), (C2, e01)):
                    nc.vector.tensor_tensor(out=sq[:], in0=ee[:], in1=ee[:], op=AluOp.mult)
                    nc.vector.tensor_reduce(
                        out=dsq[:], in_=sq[:], axis=mybir.AxisListType.X, op=AluOp.add
                    )
                # 16*area^2 = 4*A2*B2 - (A2+B2-C2)^2
                sAB = wp.tile([P, FK], F32, tag="sAB")
                nc.vector.tensor_tensor(out=sAB[:], in0=A2[:], in1=B2[:], op=AluOp.add)
                X = wp.tile([P, FK], F32, tag="X")
                nc.vector.tensor_tensor(out=X[:], in0=sAB[:], in1=C2[:], op=AluOp.subtract)
                nc.vector.tensor_tensor(out=X[:], in0=X[:], in1=X[:], op=AluOp.mult)
                disc = wp.tile([P, FK], F32, tag="disc")
                nc.vector.tensor_tensor(out=disc[:], in0=A2[:], in1=B2[:], op=AluOp.mult)
                nc.vector.tensor_scalar(
                    out=disc[:], in0=disc[:], scalar1=4.0, scalar2=None, op0=AluOp.mult
                )
                nc.vector.tensor_tensor(out=disc[:], in0=disc[:], in1=X[:], op=AluOp.subtract)
                nc.vector.tensor_scalar(
                    out=disc[:], in0=disc[:], scalar1=AREA_EPS, scalar2=None, op0=AluOp.max
                )
                inv4a = wp.tile([P, FK], F32, tag="inv4a")
                nc.scalar.activation(out=inv4a[:], in_=disc[:], func=ActFn.Sqrt)
                nc.vector.reciprocal(out=inv4a[:], in_=inv4a[:])
                # w* = cot*/4
                sumall = wp.tile([P, FK], F32, tag="sumall")
                nc.vector.tensor_tensor(out=sumall[:], in0=sAB[:], in1=C2[:], op=AluOp.add)
                wabc = []
                for nm, D2 in (("wa", A2), ("wb", B2), ("wc", C2)):
                    wt = wp.tile([P, FK], F32, tag=nm, name=nm)
                    nc.vector.tensor_scalar(
                        out=wt[:], in0=D2[:], scalar1=-2.0, scalar2=None, op0=AluOp.mult
                    )
                    nc.vector.tensor_tensor(out=wt[:], in0=wt[:], in1=sumall[:], op=AluOp.add)
                    nc.vector.tensor_tensor(out=wt[:], in0=wt[:], in1=inv4a[:], op=AluOp.mult)
                    wabc.append(wt)
                wa, wb, wc = wabc
                # scatter rows: to a: wc*vb + wb*vc | wb+wc   (cyclic)
                verts = (v0, v1, v2)
                for s, (wx, wy, vx, vy) in enumerate(
                    ((wc, wb, 1, 2), (wc, wa, 0, 2), (wb, wa, 0, 1))
                ):
                    dst3 = sval[s][:, :, ch]
                    tmp3 = wp.tile([P, FK, 3], F32, tag="tmp3")
                    nc.vector.tensor_tensor(
                        out=dst3,
                        in0=wx[:, :, None].to_broadcast([P, FK, 3]),
                        in1=verts[vx],
                        op=AluOp.mult,
                    )
                    nc.vector.tensor_tensor(
                        out=tmp3[:],
                        in0=wy[:, :, None].to_broadcast([P, FK, 3]),
                        in1=verts[vy],
                        op=AluOp.mult,
                    )
                    nc.vector.tensor_tensor(out=dst3, in0=dst3, in1=tmp3[:], op=AluOp.add)
                    nc.vector.tensor_tensor(
                        out=sval[s][:, :, 4 * b + 3 : 4 * b + 4],
                        in0=wx[:, :, None],
                        in1=wy[:, :, None],
                        op=AluOp.add,
                    )

            # scatter-add the three streams (collision-free expanded slots)
            if do_mesh:
                acc8s = [a_[:].rearrange("(a b) -> a b", b=8) for a_ in accs]
                sts = []
                for s in range(3):
                    st = cp.tile([P, FK], I32, tag=f"sidx{s}", name=f"sidx{s}t")
                    nc.sync.dma_start(out=st[:], in_=sidx[s].ap())
                    sts.append(st)
                for k in range(FK):
                    for s in range(3):
                        nc.gpsimd.indirect_dma_start(
                            out=acc8s[s],
                            out_offset=bass.IndirectOffsetOnAxis(
                                ap=sts[s][:, k : k + 1], axis=0
                            ),
                            in_=sval[s][:, k, :],
                            in_offset=None,
                            compute_op=AluOp.add,
                        )

            # ---- normal consistency ----
            for b in ((0, 1) if do_mesh else ()):
                ch = slice(4 * b, 4 * b + 3)
                e_ = wp.tile([P, PK, 3], F32, tag="nce")
                a_ = wp.tile([P, PK, 3], F32, tag="nca")
                b_ = wp.tile([P, PK, 3], F32, tag="ncb")
                nc.vector.tensor_tensor(out=e_[:], in0=pv[1][:, :, ch], in1=pv[0][:, :, ch], op=AluOp.subtract)
                nc.vector.tensor_tensor(out=a_[:], in0=pv[2][:, :, ch], in1=pv[0][:, :, ch], op=AluOp.subtract)
                nc.vector.tensor_tensor(out=b_[:], in0=pv[3][:, :, ch], in1=pv[0][:, :, ch], op=AluOp.subtract)
                n0 = wp.tile([P, PK, 3], F32, tag="n0")
                n1 = wp.tile([P, PK, 3], F32, tag="n1")
                tc3 = wp.tile([P, PK, 3], F32, tag="tc3")
                for nt, u, v in ((n0, e_, a_), (n1, e_, b_)):
                    # cross(u, v): [u1v2-u2v1, u2v0-u0v2, u0v1-u1v0]
                    for i in range(3):
                        j, k = (i + 1) % 3, (i + 2) % 3
                        nc.vector.tensor_tensor(
                            out=nt[:, :, i : i + 1],
                            in0=u[:, :, j : j + 1], in1=v[:, :, k : k + 1], op=AluOp.mult,
                        )
                        nc.vector.tensor_tensor(
                            out=tc3[:, :, i : i + 1],
                            in0=u[:, :, k : k + 1], in1=v[:, :, j : j + 1], op=AluOp.mult,
                        )
                    nc.vector.tensor_tensor(out=nt[:], in0=nt[:], in1=tc3[:], op=AluOp.subtract)
                dotn = wp.tile([P, PK], F32, tag="dotn")
                nn0 = wp.tile([P, PK], F32, tag="nn0")
                nn1 = wp.tile([P, PK], F32, tag="nn1")
                for o_, i0, i1 in ((dotn, n0, n1), (nn0, n0, n0), (nn1, n1, n1)):
                    nc.vector.tensor_tensor(out=tc3[:], in0=i0[:], in1=i1[:], op=AluOp.mult)
                    nc.vector.tensor_reduce(
                        out=o_[:], in_=tc3[:], axis=mybir.AxisListType.X, op=AluOp.add
                    )
                for nn in (nn0, nn1):
                    nc.scalar.activation(out=nn[:], in_=nn[:], func=ActFn.Sqrt)
                    nc.vector.tensor_scalar(
                        out=nn[:], in0=nn[:], scalar1=1e-8, scalar2=None, op0=AluOp.max
                    )
                den = wp.tile([P, PK], F32, tag="den")
                nc.vector.tensor_tensor(out=den[:], in0=nn0[:], in1=nn1[:], op=AluOp.mult)
                nc.vector.reciprocal(out=den[:], in_=den[:])
                # contrib = 1 - cos = 1 + dot(n0, cross(e,b)) / den   (n1_ref = -n1)
                nc.vector.tensor_tensor(out=dotn[:], in0=dotn[:], in1=den[:], op=AluOp.mult)
                nc.vector.tensor_scalar(
                    out=dotn[:], in0=dotn[:], scalar1=1.0, scalar2=None, op0=AluOp.add
                )
                nc.vector.tensor_tensor(out=dotn[:], in0=dotn[:], in1=pmask_t[:], op=AluOp.mult)
                nc.vector.tensor_reduce(
                    out=scal8[:, 3 + b : 4 + b], in_=dotn[:],
                    axis=mybir.AxisListType.X, op=AluOp.add,
                )

            # ---- reduce lap accumulator -> per-vertex sums ----
            VB = VROWS // P
            vsum = cp.tile([P, VB, 8], F32, tag="vsum")
            for g0 in (range(0, VB, 4) if do_mesh else ()):
                gn = min(4, VB - g0)
                vps = []
                for s in range(3):
                    accr = accs[s][:].rearrange("(vb p k) -> p vb k", p=P, k=SLOT * 8)
                    at = wp.tile([P, 4, SLOT * 8], F32, tag=f"accrd{s}", name=f"accrd{s}")
                    nc.sync.dma_start(out=at[:, :gn, :], in_=accr[:, g0 : g0 + gn, :])
                    vp = wp.tile([P, 4, 8], F32, tag=f"vp{s}", name=f"vp{s}")
                    nc.vector.tensor_reduce(
                        out=vp[:, :gn, :],
                        in_=at[:, :gn, :].rearrange("p a (s c) -> p a c s", c=8),
                        axis=mybir.AxisListType.X,
                        op=AluOp.add,
                    )
                    vps.append(vp)
                nc.vector.tensor_tensor(
                    out=vps[0][:, :gn, :], in0=vps[0][:, :gn, :], in1=vps[1][:, :gn, :],
                    op=AluOp.add,
                )
                nc.vector.tensor_tensor(
                    out=vsum[:, g0 : g0 + gn, :], in0=vps[0][:, :gn, :],
                    in1=vps[2][:, :gn, :], op=AluOp.add,
                )
            if do_mesh:
                nc.sync.dma_start(
                    out=ovsum.ap().rearrange("(vb p) c -> p vb c", p=P), in_=vsum[:]
                )

            # ---- final: sum scal8 over partitions via ones-matmul ----
            ones = cp.tile([P, 1], F32, tag="ones")
            nc.gpsimd.memset(ones[:], 1.0)
            with tc.tile_pool(name="psum2", bufs=1, space="PSUM") as pp2:
                psf = pp2.tile([8, 1], F32, tag="psf")
                nc.tensor.matmul(out=psf[:], lhsT=scal8[:], rhs=ones[:], start=True, stop=True)
                so = cp.tile([8, 1], F32, tag="so")
                nc.vector.tensor_copy(out=so[:], in_=psf[:])
                nc.sync.dma_start(out=oscal.ap(), in_=so[:])

            if repeat > 1:
                rep_ctx.__exit__(None, None, None)

    nc.compile()
    return nc


# --------------------------------------------------------------------------
# host-side prep
# --------------------------------------------------------------------------


def _split16(a):
    dt = _np_mm_dt()
    hi = a.astype(dt)
    lo = (a - hi.astype(np.float32)).astype(dt)
    return hi, lo


def _aug_q(q, NQP):
    n = q.shape[0]
    L = np.zeros((5, NQP), np.float32)
    L[0:3, :n] = q.T
    L[3, :n] = (q * q).sum(-1)
    L[4, :n] = 1.0
    return L


def _aug_k(k, NKP):
    m = k.shape[0]
    R = np.zeros((5, NKP), np.float32)
    R[0:3, :m] = k.T
    R[3, :] = -0.5
    R[4, :m] = -0.5 * (k * k).sum(-1)
    R[4, m:] = -BIGNEG
    return R


def _wrap128(a, K, pad_val=0):
    """[n, ...] -> [128, K, ...] with element e at (e % 128, e // 128)."""
    n = a.shape[0]
    out = np.full((K * P,) + a.shape[1:], pad_val, a.dtype)
    out[:n] = a
    return out.reshape(K, P, *a.shape[1:]).swapaxes(0, 1).copy()


def make_in_maps(inputs, cfg):
    pred = np.asarray(inputs["predictions"], np.float32)
    tgt = np.asarray(inputs["targets"], np.float32)
    faces = np.asarray(inputs["pred_faces"], np.int64)
    edges = np.asarray(inputs["edges"], np.int64)
    prs = np.asarray(inputs["nc_pairs"], np.int64)

    n = cfg["n"]
    NQP, NKP, RT = cfg["NQP"], cfg["NKP"], cfg["RT"]
    dpred = pred[:, 1:] - pred[:, :-1]
    dtgt = tgt[:, 1:] - tgt[:, :-1]

    # chamfer tasks: (queries, keys, n_valid_queries)
    tasks = []
    for b in (0, 1):
        tasks.append((pred[b], tgt[b]))
        tasks.append((tgt[b], pred[b]))
    for b in (0, 1):
        tasks.append((dpred[b], dtgt[b]))
        tasks.append((dtgt[b], dpred[b]))
    # core order: 0..3 pos (b0A, b0B, b1A, b1B), 4..7 vel
    order = [0, 1, 2, 3, 4, 5, 6, 7]

    vtab = np.zeros((n, 8), np.float32)
    vtab[:, 0:3] = pred[0]
    vtab[:, 4:7] = pred[1]

    in_maps = []
    for c in range(NCORES):
        q, k = tasks[order[c]]
        nq = q.shape[0]
        L = _aug_q(q, NQP)
        R = _aug_k(k, NKP)
        lhs_hi, lhs_lo = _split16(L)
        rhs_hi, rhs_lo = _split16(R)
        lhs15 = np.concatenate([lhs_hi, lhs_lo, lhs_hi], axis=0)
        rhs15 = np.concatenate([rhs_hi, rhs_hi, rhs_lo], axis=0)
        qm = (np.arange(NQP) < nq).astype(np.float32)
        qmask = qm.reshape(RT, P).T.copy()

        # mesh slices
        def slc(arr, per, total):
            lo = min(c * per, total)
            hi = min((c + 1) * per, total)
            return arr[lo:hi]

        fsl = slc(faces, cfg["FPC"], cfg["f"])
        esl = slc(edges, cfg["EPC"], cfg["e"])
        psl = slc(prs, cfg["PPC"], cfg["pr"])
        nf, ne, np_ = len(fsl), len(esl), len(psl)

        m = dict(lhs15=lhs15, rhs15=rhs15, qmask=qmask, vtab=vtab)
        for s in range(3):
            m[f"fidx{s}"] = _wrap128(fsl[:, s].astype(np.int32), cfg["FK"])
        for s in range(2):
            m[f"eidx{s}"] = _wrap128(esl[:, s].astype(np.int32), cfg["EK"])
        for s in range(4):
            m[f"pidx{s}"] = _wrap128(psl[:, s].astype(np.int32), cfg["PK"])
        m["emask"] = _wrap128(np.ones(ne, np.float32), cfg["EK"])
        m["pmask"] = _wrap128(np.ones(np_, np.float32), cfg["PK"])

        # collision-free expanded scatter slots (per-stream accumulators)
        fkn = cfg["FK"] * P
        SLOT = cfg["slot"]
        for s in range(3):
            tg = np.full(fkn, -1, np.int64)
            tg[:nf] = fsl[:, s]
            count = np.zeros(n, np.int64)
            dump = 0
            out = np.zeros(fkn, np.int64)
            for i_, v in enumerate(tg):
                if v < 0:
                    out[i_] = n * SLOT + dump
                    dump += 1
                else:
                    out[i_] = v * SLOT + count[v]
                    count[v] += 1
            assert count.max() <= SLOT, f"slot overflow: {count.max()} > {SLOT}"
            assert n * SLOT + dump <= cfg["ACCROWS"], "dump zone overflow"
            m[f"sidx{s}"] = _wrap128(out.astype(np.int32), cfg["FK"])

        in_maps.append(m)
    return in_maps


def combine(outs, inputs, cfg):
    pred = np.asarray(inputs["predictions"], np.float32)
    n = cfg["n"]
    scal = np.stack([o["oscal"][:, 0] for o in outs])  # [8, 8]
    # chamfer: cores 0-3 pos (n points), 4-7 vel (n-1 points)
    cham_pos = 0.5 * (scal[0, 0] + scal[1, 0] + scal[2, 0] + scal[3, 0]) / n
    cham_vel = 0.5 * (scal[4, 0] + scal[5, 0] + scal[6, 0] + scal[7, 0]) / (n - 1)
    edge_l = scal[:, 1:3].sum() / (2 * cfg["e"])
    nc_l = scal[:, 3:5].sum() / (2 * cfg["pr"])

    vs = np.zeros((n, 8), np.float64)
    for o in outs:
        vs += o["ovsum"][:n].astype(np.float64)
    lap = 0.0
    for b in (0, 1):
        Lx = vs[:, 4 * b : 4 * b + 3]
        w = vs[:, 4 * b + 3]
        nw = np.where(w > 0, 1.0 / np.where(w > 0, w, 1.0), 0.0)
        res = Lx * nw[:, None] - pred[b].astype(np.float64)
        lap += np.sqrt((res * res).sum(-1)).mean()
    lap *= 0.5

    return np.float32(
        cham_pos + W_LAP * lap + W_NORMAL * nc_l + W_EDGE * edge_l + W_VEL * cham_vel
    )


# --------------------------------------------------------------------------
# execution (cached program + cached PJRT executable)
# --------------------------------------------------------------------------

_CACHE = {}


def _get_program(dims_key):
    if dims_key not in _CACHE:
        cfg = _cfg(dict(zip(("n", "f", "e", "pr", "slot"), dims_key)))
        nc = build_program(cfg)
        _CACHE[dims_key] = (cfg, nc, {})
    return _CACHE[dims_key]


def get_runner(dims=None):
    """Returns (cfg, run_fn) where run_fn(in_maps) -> list[dict] per core."""
    import jax
    from concourse import bass2jax

    dims = dims or FULL_DIMS
    dims_key = (dims["n"], dims["f"], dims["e"], dims["pr"], dims["slot"])
    cfg, nc, aux = _get_program(dims_key)
    if "run" in aux:
        return cfg, aux["run"]

    bass2jax.install_neuronx_cc_hook()
    partition_name = nc.partition_id_tensor.name if nc.partition_id_tensor else None
    in_names, out_names, out_avals, zero_outs = [], [], [], []
    for alloc in nc.m.functions[0].allocations:
        if not isinstance(alloc, mybir.MemoryLocationSet):
            continue
        name = alloc.memorylocations[0].name
        if alloc.kind == "ExternalInput":
            if name != partition_name:
                in_names.append(name)
        elif alloc.kind == "ExternalOutput":
            shape = tuple(alloc.tensor_shape)
            dtype = mybir.dt.np(alloc.dtype)
            out_names.append(name)
            out_avals.append(jax.core.ShapedArray(shape, dtype))
            zero_outs.append(np.zeros(shape, dtype))
    n_params, n_outs = len(in_names), len(out_avals)
    all_names = in_names + out_names + ([partition_name] if partition_name else [])

    def _body(*args):
        operands = list(args)
        if partition_name is not None:
            operands.append(bass2jax.partition_id_tensor())
        return tuple(
            bass2jax._bass_exec_p.bind(
                *operands,
                out_avals=tuple(out_avals),
                in_names=tuple(all_names),
                out_names=tuple(out_names),
                lowering_input_output_aliases=(),
                sim_require_finite=True,
                sim_require_nnan=True,
                nc=nc,
            )
        )

    devices = jax.devices()[:NCORES]
    mesh = bass2jax.Mesh(np.asarray(devices), ("core",))
    PSpec = bass2jax.PartitionSpec
    sharded = jax.jit(
        bass2jax.shard_map(
            _body,
            mesh=mesh,
            in_specs=(PSpec("core"),) * (n_params + n_outs),
            out_specs=(PSpec("core"),) * n_outs,
            check_rep=False,
        ),
        donate_argnums=tuple(range(n_params, n_params + n_outs)),
        keep_unused=True,
    )

    def run(in_maps):
        concat_in = [
            np.concatenate([np.asarray(m[nm]) for m in in_maps], axis=0)
            for nm in in_names
        ]
        concat_zeros = [
            np.zeros((NCORES * z.shape[0], *z.shape[1:]), z.dtype) for z in zero_outs
        ]
        out_arrs = sharded(*concat_in, *concat_zeros)
        return [
            {
                nm: np.asarray(out_arrs[i]).reshape(NCORES, *out_avals[i].shape)[c]
                for i, nm in enumerate(out_names)
            }
            for c in range(NCORES)
        ]

    aux["run"] = run
    return cfg, run


def run_sim(in_maps, dims=None):
    """CoreSim path (no hardware) for validation."""
    from concourse.bass_interp import MultiCoreSim

    dims = dims or FULL_DIMS
    dims_key = (dims["n"], dims["f"], dims["e"], dims["pr"], dims["slot"])
    cfg, nc, _ = _get_program(dims_key)
    sim = MultiCoreSim(nc, num_cores=NCORES, trace=False)
    cores = list(sim.cores.values())
    for c, core in enumerate(cores):
        for nm, arr in in_maps[c].items():
            core.tensor(nm)[:] = arr
        for nm, shape in (("oscal", (8, 1)), ("ovsum", (cfg["VROWS"], 8))):
            core.tensor(nm)[:] = np.zeros(shape, np.float32)
    sim.simulate(check_with_hw=False)
    outs = []
    for core in cores:
        outs.append(
            {"oscal": np.array(core.tensor("oscal")), "ovsum": np.array(core.tensor("ovsum"))}
        )
    return outs


def kernel(**inputs) -> np.ndarray:
    cfg, run = get_runner(FULL_DIMS)
    in_maps = make_in_maps(inputs, cfg)
    outs = run(in_maps)
    return combine(outs, inputs, cfg)

